# revision 13
# baseline (speedup 1.0000x reference)
"""ClusterAttention2 Trainium2 kernel.

Mathematical simplification: the reference computes
    logits       : [n_clusters, 1]
    att_clusters = softmax(logits, axis=1)   # axis of size 1 -> exactly ones
    att_vertices = adj.T @ att_clusters      # == per-vertex column sum of adj
    att_vertices = att_vertices / max(att_vertices, axis=1)  # [N,1] -> x/x
so for any finite logits the output is exactly
    att_clusters = ones([n_clusters, 1])
    att_vertices = colsum / colsum           # 1.0, or NaN where colsum == 0
The only data-dependent work is the column sum of adj (400 MB -> memory
bound).  Each of the 8 cores reads a [1000, 12500] vertex-shard of adj and
reduces the cluster dimension on the tensor engine (ones[128,1].T @
adj_tile[128,500], accumulated over the 8 cluster chunks in PSUM).  The
final x/x division (IEEE 0/0 -> NaN) runs on the host so NaN positions
match the reference bit-for-bit.

Written in raw Bass (explicit semaphores): the TPB ISA allows a single
semaphore wait per instruction, so every wait is a standalone wait_ge on
the consuming engine, never attached to a data instruction.

Pipeline (per core), vertex blocks vb = 0..4 of width 2500:
  SP   : input DMAs adj[k*128:.., vb*2500:..] -> sbuf slot (vb%2, k),
         gated on s_pe so a slot is only overwritten after consumption;
         then the 5 output DMAs, gated on s_cp.
  PE   : 25 accumulation groups (vb, i): 8 matmuls each, start/stop in
         PSUM bank i%4; first use of tile (vb, k) waits s_in >= done
         count (DMA completion-order safety is guaranteed by the s_pe
         gating of future DMAs: when PE waits for tile (vb, k), no DMA
         of block vb+1 can have started).
  DVE  : memset ones; copy each finished PSUM group into obuf[vb%2].
"""

import numpy as np

import concourse.bass as bass
import concourse.mybir as mybir
from concourse.bass_utils import run_bass_kernel_spmd

N_CLUSTERS = 1000
N_VERTICES = 100000
N_CORES = 8
V_SHARD = N_VERTICES // N_CORES  # 12500 vertices per core
P = 128                          # cluster chunk (partition dim)
N_K = (N_CLUSTERS + P - 1) // P  # 8 chunks: 7x128 + 104
F_DMA = 2500                     # vertices per DMA tile (128x2500 f32 = 1.25 MB)
F_MM = 500                       # vertices per matmul (PSUM bank = 512 f32)
N_VB = V_SHARD // F_DMA          # 5 vertex blocks
N_I = F_DMA // F_MM              # 5 accumulation groups per block
N_PS = 4                         # rotating PSUM banks


def _build_nc() -> bass.Bass:
    nc = bass.Bass()
    adj_s = nc.dram_tensor(
        "adj_s", [N_CLUSTERS, V_SHARD], mybir.dt.float32, kind="ExternalInput"
    )
    cs = nc.dram_tensor("cs", [V_SHARD], mybir.dt.float32, kind="ExternalOutput")

    with (
        nc.sbuf_tensor([P, 2 * N_K * F_DMA], mybir.dt.float32) as tbuf,
        nc.sbuf_tensor([P, 1], mybir.dt.float32) as ones,
        nc.sbuf_tensor([1, 2 * F_DMA], mybir.dt.float32) as obuf,
        # 512-f32 stride so each rotating accumulator is bank-aligned
        nc.psum_tensor([1, N_PS, 512], mybir.dt.float32) as pst,
        nc.semaphore("s_init") as s_init,
        nc.semaphore("s_in") as s_in,
        nc.semaphore("s_pe") as s_pe,
        nc.semaphore("s_cp") as s_cp,
        nc.semaphore("s_out") as s_out,
        nc.Block() as block,
    ):

        def tslot(vb, k):
            return tbuf[:, ((vb % 2) * N_K + k) * F_DMA : ((vb % 2) * N_K + k + 1) * F_DMA]

        def kp_of(k):
            return min(P, N_CLUSTERS - k * P)

        @block.sync
        def _(sync):
            for vb in range(N_VB):
                if vb >= 2:
                    # slots (vb%2, *) free once PE consumed all of block vb-2
                    sync.wait_ge(s_pe, N_I * (vb - 1))
                for k in range(N_K):
                    kp = kp_of(k)
                    sync.dma_start(
                        out=tslot(vb, k)[:kp, :],
                        in_=adj_s[k * P : k * P + kp, vb * F_DMA : (vb + 1) * F_DMA],
                    ).then_inc(s_in, 16)

        @block.scalar
        def _(scalar):
            for vb in range(N_VB):
                scalar.wait_ge(s_cp, N_I * (vb + 1))
                scalar.dma_start(
                    out=cs[vb * F_DMA : (vb + 1) * F_DMA],
                    in_=obuf[:1, (vb % 2) * F_DMA : (vb % 2 + 1) * F_DMA],
                ).then_inc(s_out, 16)
            scalar.wait_ge(s_out, 16 * N_VB)

        @block.tensor
        def _(tensor):
            tensor.wait_ge(s_init, 1)  # ones ready
            for vb in range(N_VB):
                for i in range(N_I):
                    g = vb * N_I + i
                    if g >= N_PS:
                        # PSUM bank g%N_PS free once copy of group g-N_PS done
                        tensor.wait_ge(s_cp, g - N_PS + 1)
                    for k in range(N_K):
                        kp = kp_of(k)
                        if i == 0:
                            # first use of tile (vb, k)
                            tensor.wait_ge(s_in, 16 * (N_K * vb + k + 1))
                        mm = nc.tensor.matmul(
                            pst[:1, g % N_PS, :F_MM],
                            ones[:kp, :1],
                            tslot(vb, k)[:kp, i * F_MM : (i + 1) * F_MM],
                            start=(k == 0),
                            stop=(k == N_K - 1),
                        )
                        if k == N_K - 1:
                            mm.then_inc(s_pe, 1)

        @block.vector
        def _(vector):
            vector.memset(ones[:, :], 1.0).then_inc(s_init, 1)
            for vb in range(N_VB):
                for i in range(N_I):
                    g = vb * N_I + i
                    if i == 0 and vb >= 2:
                        # obuf half (vb%2) free once output DMA of vb-2 done
                        vector.wait_ge(s_out, 16 * (vb - 1))
                    vector.wait_ge(s_pe, g + 1)
                    nc.vector.tensor_copy(
                        obuf[:1, (vb % 2) * F_DMA + i * F_MM : (vb % 2) * F_DMA + (i + 1) * F_MM],
                        pst[:1, g % N_PS, :F_MM],
                    ).then_inc(s_cp, 1)

    return nc


def kernel(x, adj, att, key_w):
    adj = np.ascontiguousarray(np.asarray(adj), dtype=np.float32)
    assert adj.shape == (N_CLUSTERS, N_VERTICES)

    nc = _build_nc()
    in_maps = [
        {"adj_s": np.ascontiguousarray(adj[:, i * V_SHARD : (i + 1) * V_SHARD])}
        for i in range(N_CORES)
    ]
    res = run_bass_kernel_spmd(nc, in_maps, core_ids=list(range(N_CORES)))
    colsum = np.concatenate([r["cs"] for r in res.results]).astype(np.float32)

    with np.errstate(divide="ignore", invalid="ignore"):
        att_vertices = (colsum / colsum).reshape(N_VERTICES, 1).astype(np.float32)
    att_clusters = np.ones((N_CLUSTERS, 1), dtype=np.float32)
    return att_vertices, att_clusters
